# revision 7
# baseline (speedup 1.0000x reference)
"""BLOutputLayer forward: out[i] = features[rules[i]] — a rule-book gather.

Strategy (8 NeuronCores, data-parallel over output rows):
  - rules [524288] -> 8 shards of 65536 rows, one per core; features
    [200000, 64] f32 replicated to every core's DRAM.

  v2 (fast path): the int16-indexed SWDGE gather/scatter instructions
  (InstDMAGatherAnt / InstDMAScatterAddAnt) move one 256 B row per
  descriptor with only ~1 us fixed cost per *instruction*, so we want
  few instructions with many indices each. int16 limits reach to 32768
  rows, so the host buckets each core's (feature_idx, out_pos) pairs by
  (feature chunk of 32768 rows x output group of 32768 rows) = 14
  buckets. Per bucket: dma_gather (chunk-local idx) -> SBUF tile in
  [i%128, i//128] layout -> dma_scatter_add (group-local pos) into the
  pre-zeroed DRAM output (add == write). Host work touches only the
  4 MB index array; all 268 MB of data movement stays on device.

  v1 (simple fallback): 512 indirect DMAs of 128 rows (one index per
  partition), ~1.5 us each -> ~758 us/core. Kept for reference.
"""

import numpy as np

import concourse.bacc as bacc
import concourse.mybir as mybir
import concourse.tile as tile
from concourse.bass import IndirectOffsetOnAxis
from concourse.bass_utils import run_bass_kernel_spmd

N_ACTIVE = 200000
C = 64
N_ROWS = 524288
N_CORES = 8
ROWS_PER_CORE = N_ROWS // N_CORES  # 65536
P = 128

CHUNK = 32768  # feature rows addressable by int16 gather indices
N_CHUNKS = (N_ACTIVE + CHUNK - 1) // CHUNK  # 7 (last chunk 3392 rows)
GRP = 32768  # output rows addressable by int16 scatter indices
N_GRP = ROWS_PER_CORE // GRP  # 2
N_BUCKETS = N_CHUNKS * N_GRP  # 14

_cache = {}


def _wrap16(a):
    """[S] -> [128, S//16] int16 in the SWDGE wrapped layout: entry j at
    (j % 16, j // 16), replicated across the eight 16-partition groups."""
    w = a.reshape(-1, 16).T  # [16, S//16]
    return np.tile(w, (8, 1)).copy()


def plan_v2(rules_i32):
    """Bucket (idx, pos) pairs per core; returns static sizes + host arrays."""
    shards = rules_i32.reshape(N_CORES, ROWS_PER_CORE).astype(np.int64)
    pos = np.arange(ROWS_PER_CORE, dtype=np.int64)
    per_core = []
    counts_all = np.zeros((N_CORES, N_BUCKETS), dtype=np.int64)
    for c in range(N_CORES):
        idx = shards[c]
        key = (idx >> 15) * N_GRP + (pos >> 15)
        order = np.argsort(key, kind="stable")
        counts_all[c] = np.bincount(key, minlength=N_BUCKETS)
        per_core.append((idx[order], pos[order]))
    # shared static per-bucket sizes: max over cores, rounded up to 128
    S = np.maximum(((counts_all.max(axis=0) + 127) // 128) * 128, 128).astype(int)
    tot_cols = int(S.sum()) // 16

    gidx_w = np.empty((N_CORES, P, tot_cols), dtype=np.int16)
    sidx_w = np.empty((N_CORES, P, tot_cols), dtype=np.int16)
    cnts = np.zeros((N_CORES, 1, N_BUCKETS), dtype=np.int32)
    for c in range(N_CORES):
        idx_s, pos_s = per_core[c]
        counts = counts_all[c]
        starts = np.concatenate([[0], np.cumsum(counts)])
        col = 0
        for b in range(N_BUCKETS):
            chunk, grp = divmod(b, N_GRP)
            n, s_b = int(counts[b]), int(S[b])
            g = np.full(s_b, -1, dtype=np.int16)
            s_ = np.full(s_b, -1, dtype=np.int16)
            sel = slice(starts[b], starts[b] + n)
            g[:n] = (idx_s[sel] - chunk * CHUNK).astype(np.int16)
            s_[:n] = (pos_s[sel] - grp * GRP).astype(np.int16)
            w = s_b // 16
            gidx_w[c, :, col : col + w] = _wrap16(g)
            sidx_w[c, :, col : col + w] = _wrap16(s_)
            cnts[c, 0, b] = n
            col += w
    return tuple(S.tolist()), gidx_w, sidx_w, cnts


def build_v2(S, reps=1):
    nc = bacc.Bacc("TRN2", target_bir_lowering=False)
    tot_cols = sum(S) // 16
    features = nc.dram_tensor(
        "features", [N_ACTIVE, C], mybir.dt.float32, kind="ExternalInput"
    )
    gidx = nc.dram_tensor("gidx", [P, tot_cols], mybir.dt.int16, kind="ExternalInput")
    sidx = nc.dram_tensor("sidx", [P, tot_cols], mybir.dt.int16, kind="ExternalInput")
    cnt = nc.dram_tensor("cnt", [1, N_BUCKETS], mybir.dt.int32, kind="ExternalInput")
    out = nc.dram_tensor(
        "out", [ROWS_PER_CORE, C], mybir.dt.float32, kind="ExternalOutput"
    )

    with tile.TileContext(nc) as tc:
        with (
            tc.tile_pool(name="idx", bufs=1) as idx_pool,
            tc.tile_pool(name="data", bufs=3) as data_pool,
        ):
            gidx_t = idx_pool.tile([P, tot_cols], mybir.dt.int16, tag="gidx")
            sidx_t = idx_pool.tile([P, tot_cols], mybir.dt.int16, tag="sidx")
            cnt_t = idx_pool.tile([1, N_BUCKETS], mybir.dt.int32, tag="cnt")
            nc.sync.dma_start(out=gidx_t[:], in_=gidx[:])
            nc.sync.dma_start(out=sidx_t[:], in_=sidx[:])
            nc.sync.dma_start(out=cnt_t[:], in_=cnt[:])
            regs = [
                nc.alloc_register(mybir.EngineType.Pool, f"cnt{b}")
                for b in range(N_BUCKETS)
            ]
            for b in range(N_BUCKETS):
                nc.gpsimd.reg_load(regs[b], cnt_t[:1, b : b + 1])
            for _rep in range(reps):
                col = 0
                for b in range(N_BUCKETS):
                    chunk, grp = divmod(b, N_GRP)
                    s_b = S[b]
                    w = s_b // 16
                    c_end = min((chunk + 1) * CHUNK, N_ACTIVE)
                    data_t = data_pool.tile(
                        [P, s_b // 128, C], mybir.dt.float32, tag="data"
                    )
                    nc.vector.memset(data_t[:], 0)
                    nc.gpsimd.dma_gather(
                        data_t[:],
                        features[chunk * CHUNK : c_end],
                        gidx_t[:, col : col + w],
                        num_idxs=s_b,
                        num_idxs_reg=regs[b],
                        elem_size=C,
                        elem_step=C,
                        single_packet=False,
                    )
                    nc.gpsimd.dma_scatter_add(
                        out[grp * GRP : (grp + 1) * GRP],
                        data_t[:],
                        sidx_t[:, col : col + w],
                        num_idxs=s_b,
                        num_idxs_reg=regs[b],
                        elem_size=C,
                        elem_step=C,
                        single_packet=False,
                    )
                    col += w
    nc.finalize()
    return nc


def run(features, rules, reps=1):
    features = np.ascontiguousarray(np.asarray(features), dtype=np.float32)
    rules_i32 = np.ascontiguousarray(np.asarray(rules)).astype(np.int32)

    S, gidx_w, sidx_w, cnts = plan_v2(rules_i32)
    key = ("v2", S, reps)
    if _cache.get("key") != key:
        _cache["nc"] = build_v2(S, reps)
        _cache["key"] = key
    nc = _cache["nc"]

    in_maps = [
        {"features": features, "gidx": gidx_w[c], "sidx": sidx_w[c], "cnt": cnts[c]}
        for c in range(N_CORES)
    ]
    res = run_bass_kernel_spmd(nc, in_maps, list(range(N_CORES)))
    full = np.concatenate([res.results[c]["out"] for c in range(N_CORES)], axis=0)
    return full, res


def kernel(**inputs):
    full, _ = run(inputs["features"], inputs["rules"])
    return full


# ---------------------------------------------------------------------------
# v1 (simple indirect-DMA version, ~758 us/core) kept for reference/benching
N_GATHERS = ROWS_PER_CORE // P  # 512
G = 32
N_GROUPS = N_GATHERS // G  # 16


def _build(reps=1):
    nc = bacc.Bacc("TRN2", target_bir_lowering=False)
    features = nc.dram_tensor(
        "features", [N_ACTIVE, C], mybir.dt.float32, kind="ExternalInput"
    )
    rules = nc.dram_tensor(
        "rules", [P, N_GATHERS], mybir.dt.int32, kind="ExternalInput"
    )
    out = nc.dram_tensor(
        "out", [N_GROUPS, P, G, C], mybir.dt.float32, kind="ExternalOutput"
    )

    with tile.TileContext(nc) as tc:
        with (
            tc.tile_pool(name="idx", bufs=1) as idx_pool,
            tc.tile_pool(name="data", bufs=3) as data_pool,
        ):
            idx_tile = idx_pool.tile([P, N_GATHERS], mybir.dt.int32, tag="idx")
            nc.sync.dma_start(out=idx_tile[:], in_=rules[:])
            for _rep in range(reps):
                for grp in range(N_GROUPS):
                    data_tile = data_pool.tile([P, G, C], mybir.dt.float32, tag="data")
                    for g in range(G):
                        j = grp * G + g
                        nc.gpsimd.indirect_dma_start(
                            out=data_tile[:, g],
                            out_offset=None,
                            in_=features[:],
                            in_offset=IndirectOffsetOnAxis(
                                ap=idx_tile[:, j : j + 1], axis=0
                            ),
                        )
                    nc.sync.dma_start(out=out[grp], in_=data_tile[:])
    nc.finalize()
    return nc


# revision 8
# speedup vs baseline: 1.9075x; 1.9075x over previous
"""BLOutputLayer forward: out[i] = features[rules[i]] — a rule-book gather.

Strategy (8 NeuronCores, data-parallel over output rows):
  - rules [524288] -> 8 shards of 65536 rows, one per core; features
    [200000, 64] f32 replicated to every core's DRAM.

  v2 (fast path): the int16-indexed SWDGE gather/scatter instructions
  (InstDMAGatherAnt / InstDMAScatterAddAnt) move one 256 B row per
  descriptor with only ~1 us fixed cost per *instruction*, so we want
  few instructions with many indices each. int16 limits reach to 32768
  rows, so the host buckets each core's (feature_idx, out_pos) pairs by
  (feature chunk of 32768 rows x output group of 32768 rows) = 14
  buckets. Per bucket: dma_gather (chunk-local idx) -> SBUF tile in
  [i%128, i//128] layout -> dma_scatter_add (group-local pos) into the
  pre-zeroed DRAM output (add == write). Host work touches only the
  4 MB index array; all 268 MB of data movement stays on device.

  v1 (simple fallback): 512 indirect DMAs of 128 rows (one index per
  partition), ~1.5 us each -> ~758 us/core. Kept for reference.
"""

import numpy as np

import concourse.bacc as bacc
import concourse.mybir as mybir
import concourse.tile as tile
from concourse.bass import IndirectOffsetOnAxis
from concourse.bass_utils import run_bass_kernel_spmd

N_ACTIVE = 200000
C = 64
N_ROWS = 524288
N_CORES = 8
ROWS_PER_CORE = N_ROWS // N_CORES  # 65536
P = 128

CHUNK = 32768  # feature rows addressable by int16 gather indices
N_CHUNKS = (N_ACTIVE + CHUNK - 1) // CHUNK  # 7 (last chunk 3392 rows)
GRP = 32768  # output rows addressable by int16 scatter indices
N_GRP = ROWS_PER_CORE // GRP  # 2
N_BUCKETS = N_CHUNKS * N_GRP  # 14

_cache = {}


def _wrap16(a):
    """[S] -> [128, S//16] int16 in the SWDGE wrapped layout: entry j at
    (j % 16, j // 16), replicated across the eight 16-partition groups."""
    w = a.reshape(-1, 16).T  # [16, S//16]
    return np.tile(w, (8, 1)).copy()


def plan_v2(rules_i32):
    """Bucket (idx, pos) pairs per core; returns static sizes + host arrays."""
    shards = rules_i32.reshape(N_CORES, ROWS_PER_CORE).astype(np.int64)
    pos = np.arange(ROWS_PER_CORE, dtype=np.int64)
    per_core = []
    counts_all = np.zeros((N_CORES, N_BUCKETS), dtype=np.int64)
    for c in range(N_CORES):
        idx = shards[c]
        key = (idx >> 15) * N_GRP + (pos >> 15)
        order = np.argsort(key, kind="stable")
        counts_all[c] = np.bincount(key, minlength=N_BUCKETS)
        per_core.append((idx[order], pos[order]))
    # shared static per-bucket sizes: max over cores, rounded up to 128
    S = np.maximum(((counts_all.max(axis=0) + 127) // 128) * 128, 128).astype(int)
    tot_cols = int(S.sum()) // 16

    gidx_w = np.empty((N_CORES, P, tot_cols), dtype=np.int16)
    sidx_w = np.empty((N_CORES, P, tot_cols), dtype=np.int16)
    cnts = np.zeros((N_CORES, 1, N_BUCKETS), dtype=np.int32)
    for c in range(N_CORES):
        idx_s, pos_s = per_core[c]
        counts = counts_all[c]
        starts = np.concatenate([[0], np.cumsum(counts)])
        col = 0
        for b in range(N_BUCKETS):
            chunk, grp = divmod(b, N_GRP)
            n, s_b = int(counts[b]), int(S[b])
            g = np.full(s_b, -1, dtype=np.int16)
            s_ = np.full(s_b, -1, dtype=np.int16)
            sel = slice(starts[b], starts[b] + n)
            g[:n] = (idx_s[sel] - chunk * CHUNK).astype(np.int16)
            s_[:n] = (pos_s[sel] - grp * GRP).astype(np.int16)
            w = s_b // 16
            gidx_w[c, :, col : col + w] = _wrap16(g)
            sidx_w[c, :, col : col + w] = _wrap16(s_)
            cnts[c, 0, b] = n
            col += w
    return tuple(S.tolist()), gidx_w, sidx_w, cnts


def build_v2(S, reps=1):
    nc = bacc.Bacc("TRN2", target_bir_lowering=False, num_swdge_queues=4)
    tot_cols = sum(S) // 16
    features = nc.dram_tensor(
        "features", [N_ACTIVE, C], mybir.dt.float32, kind="ExternalInput"
    )
    gidx = nc.dram_tensor("gidx", [P, tot_cols], mybir.dt.int16, kind="ExternalInput")
    sidx = nc.dram_tensor("sidx", [P, tot_cols], mybir.dt.int16, kind="ExternalInput")
    cnt = nc.dram_tensor("cnt", [1, N_BUCKETS], mybir.dt.int32, kind="ExternalInput")
    out = nc.dram_tensor(
        "out", [ROWS_PER_CORE, C], mybir.dt.float32, kind="ExternalOutput"
    )

    with tile.TileContext(nc) as tc:
        with (
            tc.tile_pool(name="idx", bufs=1) as idx_pool,
            tc.tile_pool(name="data", bufs=3) as data_pool,
        ):
            gidx_t = idx_pool.tile([P, tot_cols], mybir.dt.int16, tag="gidx")
            sidx_t = idx_pool.tile([P, tot_cols], mybir.dt.int16, tag="sidx")
            cnt_t = idx_pool.tile([1, N_BUCKETS], mybir.dt.int32, tag="cnt")
            nc.sync.dma_start(out=gidx_t[:], in_=gidx[:])
            nc.sync.dma_start(out=sidx_t[:], in_=sidx[:])
            nc.sync.dma_start(out=cnt_t[:], in_=cnt[:])
            regs = [
                nc.alloc_register(mybir.EngineType.Pool, f"cnt{b}")
                for b in range(N_BUCKETS)
            ]
            for b in range(N_BUCKETS):
                nc.gpsimd.reg_load(regs[b], cnt_t[:1, b : b + 1])
            for _rep in range(reps):
                col = 0
                for b in range(N_BUCKETS):
                    chunk, grp = divmod(b, N_GRP)
                    s_b = S[b]
                    w = s_b // 16
                    c_end = min((chunk + 1) * CHUNK, N_ACTIVE)
                    data_t = data_pool.tile(
                        [P, s_b // 128, C], mybir.dt.float32, tag="data"
                    )
                    nc.vector.memset(data_t[:], 0)
                    nc.gpsimd.dma_gather(
                        data_t[:],
                        features[chunk * CHUNK : c_end],
                        gidx_t[:, col : col + w],
                        num_idxs=s_b,
                        num_idxs_reg=regs[b],
                        elem_size=C,
                        elem_step=C,
                        single_packet=False,
                        queue_num=(2 * b) % 4,
                    )
                    nc.gpsimd.dma_scatter_add(
                        out[grp * GRP : (grp + 1) * GRP],
                        data_t[:],
                        sidx_t[:, col : col + w],
                        num_idxs=s_b,
                        num_idxs_reg=regs[b],
                        elem_size=C,
                        elem_step=C,
                        single_packet=False,
                        queue_num=(2 * b + 1) % 4,
                    )
                    col += w
    nc.finalize()
    return nc


def run(features, rules, reps=1):
    features = np.ascontiguousarray(np.asarray(features), dtype=np.float32)
    rules_i32 = np.ascontiguousarray(np.asarray(rules)).astype(np.int32)

    S, gidx_w, sidx_w, cnts = plan_v2(rules_i32)
    key = ("v2", S, reps)
    if _cache.get("key") != key:
        _cache["nc"] = build_v2(S, reps)
        _cache["key"] = key
    nc = _cache["nc"]

    in_maps = [
        {"features": features, "gidx": gidx_w[c], "sidx": sidx_w[c], "cnt": cnts[c]}
        for c in range(N_CORES)
    ]
    res = run_bass_kernel_spmd(nc, in_maps, list(range(N_CORES)))
    full = np.concatenate([res.results[c]["out"] for c in range(N_CORES)], axis=0)
    return full, res


def kernel(**inputs):
    full, _ = run(inputs["features"], inputs["rules"])
    return full


# ---------------------------------------------------------------------------
# v1 (simple indirect-DMA version, ~758 us/core) kept for reference/benching
N_GATHERS = ROWS_PER_CORE // P  # 512
G = 32
N_GROUPS = N_GATHERS // G  # 16


def _build(reps=1):
    nc = bacc.Bacc("TRN2", target_bir_lowering=False)
    features = nc.dram_tensor(
        "features", [N_ACTIVE, C], mybir.dt.float32, kind="ExternalInput"
    )
    rules = nc.dram_tensor(
        "rules", [P, N_GATHERS], mybir.dt.int32, kind="ExternalInput"
    )
    out = nc.dram_tensor(
        "out", [N_GROUPS, P, G, C], mybir.dt.float32, kind="ExternalOutput"
    )

    with tile.TileContext(nc) as tc:
        with (
            tc.tile_pool(name="idx", bufs=1) as idx_pool,
            tc.tile_pool(name="data", bufs=3) as data_pool,
        ):
            idx_tile = idx_pool.tile([P, N_GATHERS], mybir.dt.int32, tag="idx")
            nc.sync.dma_start(out=idx_tile[:], in_=rules[:])
            for _rep in range(reps):
                for grp in range(N_GROUPS):
                    data_tile = data_pool.tile([P, G, C], mybir.dt.float32, tag="data")
                    for g in range(G):
                        j = grp * G + g
                        nc.gpsimd.indirect_dma_start(
                            out=data_tile[:, g],
                            out_offset=None,
                            in_=features[:],
                            in_offset=IndirectOffsetOnAxis(
                                ap=idx_tile[:, j : j + 1], axis=0
                            ),
                        )
                    nc.sync.dma_start(out=out[grp], in_=data_tile[:])
    nc.finalize()
    return nc
